# revision 4
# baseline (speedup 1.0000x reference)
"""Trainium2 Bass kernel for nn_AutoReconstruction — hybrid PE + DVE/ACT.

Computes out[b, m] = dot(inputs[b, m, :], W[m, :]) + bias[m]
  inputs: [1024, 2048, 128] f32, W: [2048, 128] f32, bias: [2048] f32
  out:    [1024, 2048] f32

HW exec ~257 us vs the 482 us fp32 DVE/ACT baseline (1.9x). How:
  - X cast to fp16 on the host (norm rel err ~2.8e-4, tolerance 2e-2),
    halving HBM traffic to ~67 MiB/core (~180 us DMA floor @375 GB/s).
  - m-space is split across two independent compute pipelines:
      * PE pairs, m in [896, 2048): host packs X^T chunk-major
        [k][i][b][ml] so each chunk DMA is one 48 KB contiguous run
        per partition (full DMA rate; a plain [I,B,M] layout's 512 B
        runs only reached 200 GB/s). Per m: ldweights X[i_p, b-cols]
        (stationary, 128 cols) + 1-col matmul with W^T[i_p, m]
        -> psum[b_p, m]. Pair rate ~90-146 ns => ~170 us for 1152 m.
        Output needs no transpose and no DVE/ACT reduction at all.
      * DVE/ACT, m in [0, 896), p-major [m_p, b, c, i]: fp16
        tensor_mul in DVE 2x mode (~80 us), i-reduction routed by
        b%32: 23/32 of batches to DVE tensor_reduce (1x, ~1.1 us/b),
        9/32 to ScalarE activation+accum (~4.7 us/b incl. the
        ACTIVATION_READ_ACCUMULATOR cost) -> both lanes ~170 us.
  - All X DMAs ride the sync queue, interleaved PE-chunk : 6 DVE
    tiles (scalar queue only carries consts — ACT must not own a
    data queue, it stalls DMA issue behind activations).
  - Epilogue: PE half = one scalar_tensor_tensor (psum*1+bias) and
    its out-DMA ships early; DVE half = broadcast bias add, 7 PE
    transposes, strided copies, second out-DMA.

Known dead ends (measured on this HW): bf16 loses the DVE 2x mult
mode (fp16 keeps it) and gets no fast reduce; Pool rejects both
tensor_scalar+accum and fp16 tensor_tensor at walrus/ISA level;
pure-PE per-m pairs cap at ~188 ns/pair => 378 us; int8/fp8 PE
matmul is blocked by the cost model / fails accuracy.

Sharding: batch dim B=1024 split across 8 NeuronCores (128 each);
weights/bias replicated.
"""

import numpy as np

B, M, I = 1024, 2048, 128
NCORES = 8
BLOC = B // NCORES   # 128 batches per core

MDVE = 896           # m in [0, 896): DVE/ACT path (p-major layout)
CD = MDVE // 128     # 7 m's per partition
MPE = M - MDVE       # 1152 m's on the PE pair path
KCH = 6              # PE x chunks
MCP = MPE // KCH     # 192 m's per PE chunk
NB = 4               # batches per DVE-path input DMA

_CACHE = {}
LAST_RESULT = None

_AXON_PJRT_SO = "/opt/axon/libaxon_pjrt.so"


def _ensure_ntff_hook():
    """Provide antenv.axon_hooks if the image lacks it (see baseline)."""
    import sys
    try:
        from antenv.axon_hooks import get_axon_ntff_profile_hook  # noqa: F401
        return
    except ImportError:
        pass
    import contextlib
    import ctypes
    import types

    hook = None
    try:
        lib = ctypes.CDLL(_AXON_PJRT_SO)
        if hasattr(lib, "axon_start_nrt_profile"):
            lib.axon_start_nrt_profile.argtypes = [
                ctypes.POINTER(ctypes.c_int64), ctypes.c_size_t]
            lib.axon_start_nrt_profile.restype = ctypes.c_int64
            lib.axon_stop_nrt_profile.argtypes = [ctypes.c_char_p]
            lib.axon_stop_nrt_profile.restype = ctypes.c_int64

            @contextlib.contextmanager
            def _hook(output_dir, device_ids):
                import jax
                jax.devices()
                if device_ids:
                    ids = (ctypes.c_int64 * len(device_ids))(*device_ids)
                    rc = lib.axon_start_nrt_profile(ids, len(device_ids))
                else:
                    rc = lib.axon_start_nrt_profile(None, 0)
                if rc != 0:
                    raise RuntimeError(f"axon_start_nrt_profile rc={rc}")
                try:
                    yield
                finally:
                    n = lib.axon_stop_nrt_profile(str(output_dir).encode())
                    if n <= 0:
                        import sys as _s
                        print(f"profile: rc={n} writing {output_dir}",
                              file=_s.stderr)

            hook = _hook
    except OSError:
        pass

    mod = types.ModuleType("antenv.axon_hooks")
    _state = {"hook": hook}
    mod.get_axon_ntff_profile_hook = lambda: _state["hook"]
    mod.set_axon_ntff_profile_hook = lambda h: _state.__setitem__("hook", h)
    sys.modules["antenv.axon_hooks"] = mod
    try:
        import antenv
        antenv.axon_hooks = mod
    except ImportError:
        pass


def _route(b):
    """Reduce engine for batch b on the DVE path: 'v' DVE, 'a' ACT.

    (Pool tensor_scalar+accum passes CoreSim but walrus rejects
    TensorScalarPtr on the Pool engine, so no Pool share.)
    """
    # Front-load ACT batches: ACT is ~4.7 us/batch, so its share must
    # finish early; the last 32 batches are all-DVE to shorten the tail.
    return "a" if (b < 96 and b % 8 < 3) else "v"


def _build_nc():
    import concourse.bass as bass  # noqa: F401
    import concourse.tile as tile
    from concourse import bacc, mybir

    f32 = mybir.dt.float32
    f16 = mybir.dt.float16
    bf16 = mybir.dt.bfloat16
    nc = bacc.Bacc("TRN2", target_bir_lowering=False, debug=False,
                   num_devices=NCORES)

    x_pe = nc.dram_tensor("x_pe", [KCH, I, BLOC, MCP], f16,
                          kind="ExternalInput").ap()
    x_dve = nc.dram_tensor("x_dve", [128, BLOC, CD, I], f16,
                           kind="ExternalInput").ap()
    wt_pe = nc.dram_tensor("wt_pe", [I, MPE], f16, kind="ExternalInput").ap()
    w_dve = nc.dram_tensor("w_dve", [128, CD, I], f16,
                           kind="ExternalInput").ap()
    bias_pe = nc.dram_tensor("bias_pe", [128, MPE], f32,
                             kind="ExternalInput").ap()
    bias_dve = nc.dram_tensor("bias_dve", [128, MDVE], f32,
                              kind="ExternalInput").ap()
    out_d = nc.dram_tensor("out", [BLOC, M], f32, kind="ExternalOutput").ap()
    ident_d = nc.inline_tensor(np.eye(128, dtype=np.float32), name="ident")

    mult = mybir.AluOpType.mult
    add = mybir.AluOpType.add
    ident_fn = mybir.ActivationFunctionType.Identity

    with tile.TileContext(nc) as tc:
        with tc.tile_pool(name="const", bufs=1) as cpool, \
             tc.tile_pool(name="xpe", bufs=2) as xppool, \
             tc.tile_pool(name="xdve", bufs=6) as xdpool, \
             tc.tile_pool(name="prodp", bufs=8) as ppool, \
             tc.tile_pool(name="scrp", bufs=6) as spool, \
             tc.tile_pool(name="accd", bufs=1) as adpool, \
             tc.tile_pool(name="outp", bufs=1) as opool, \
             tc.tile_pool(name="accpe", bufs=1, space="PSUM") as apepool, \
             tc.tile_pool(name="tpp", bufs=2, space="PSUM") as tppool:

            # ---- constants (scalar queue) ----
            wt_pe_sb = cpool.tile([I, MPE], f16, name="wt_pe_sb")
            nc.scalar.dma_start(wt_pe_sb[:], wt_pe[:])
            w_dve_sb = cpool.tile([128, CD, I], f16, name="w_dve_sb")
            nc.scalar.dma_start(w_dve_sb[:], w_dve[:])
            bias_pe_sb = cpool.tile([128, MPE], f32, name="bias_pe_sb")
            nc.scalar.dma_start(bias_pe_sb[:], bias_pe[:])
            bias_dve_sb = cpool.tile([128, MDVE], f32, name="bias_dve_sb")
            nc.scalar.dma_start(bias_dve_sb[:], bias_dve[:])
            ident_sb = cpool.tile([128, 128], f32, name="ident_sb")
            nc.scalar.dma_start(ident_sb[:], ident_d.ap())

            # ---- accumulators ----
            acc_pe = apepool.tile([128, MPE], f32, name="acc_pe")
            acc_dve = adpool.tile([128, BLOC, CD], f32, name="acc_dve")
            out_sb = opool.tile([128, M], f32, name="out_sb")

            # ---- interleave DMA issue: PE chunks (sync q), DVE tiles
            # (scalar q); compute instructions are scheduled by deps ----
            n_dve_tiles = BLOC // NB  # 32

            def pe_chunk(k):
                xt = xppool.tile([I, BLOC, MCP], f16, name="xt_pe", tag="xpe")
                nc.sync.dma_start(xt[:], x_pe[k])
                for ml in range(MCP):
                    m = k * MCP + ml
                    nc.tensor.matmul(
                        acc_pe[:, m:m + 1],
                        xt[:, :, ml],          # stationary [i_p, b]
                        wt_pe_sb[:, m:m + 1],  # moving [i_p, 1]
                        start=True, stop=True,
                    )

            def dve_tiles(t0, t1):
                for t in range(t0, t1):
                    bb = t * NB
                    xdt = xdpool.tile([128, NB, CD, I], f16, name="xt_dve",
                                      tag="xdve")
                    nc.sync.dma_start(xdt[:], x_dve[:, bb:bb + NB])
                    for j in range(NB):
                        b = bb + j
                        prod = ppool.tile([128, CD, I], f16, name="prod",
                                          tag="prod")
                        nc.vector.tensor_mul(prod[:], xdt[:, j], w_dve_sb[:])
                        if _route(b) == "v":
                            nc.vector.tensor_reduce(
                                out=acc_dve[:, b], in_=prod[:],
                                axis=mybir.AxisListType.X, op=add)
                        else:
                            for c in range(CD):
                                scr = spool.tile([128, I], f16, name="scr",
                                                 tag="scr")
                                nc.scalar.activation(
                                    out=scr[:], in_=prod[:, c], func=ident_fn,
                                    bias=0.0, scale=1.0,
                                    accum_out=acc_dve[:, b, c:c + 1])

            # issue order: alternate so both queues fill early
            tiles_per_chunk = (n_dve_tiles + KCH - 1) // KCH  # ~6
            dve_tiles(0, 2)  # prime the DVE lane before the 6 MB PE chunk
            t_done = 2
            for k in range(KCH):
                pe_chunk(k)
                t_next = min(n_dve_tiles, t_done + tiles_per_chunk)
                dve_tiles(t_done, t_next)
                t_done = t_next
            dve_tiles(t_done, n_dve_tiles)

            # ---- epilogue ----
            # PE half: out_sb[:, MDVE:] = acc_pe * 1.0 + bias_pe; ship it
            # as soon as the pairs are done (PE finishes ~25 us before the
            # DVE lane drains).
            nc.vector.scalar_tensor_tensor(
                out=out_sb[:, MDVE:M], in0=acc_pe[:], scalar=1.0,
                in1=bias_pe_sb[:], op0=mult, op1=add)
            nc.sync.dma_start(out_d[:, MDVE:M], out_sb[:, MDVE:M])

            # DVE half: transpose [m_p, b, c] -> [b_p, m], bias fused
            # into the copy (out = tp*1 + bias_rep cols)
            for c in range(CD):
                tp = tppool.tile([128, 128], f32, name="tp", tag="tp")
                nc.tensor.transpose(tp[:], acc_dve[:, :, c], ident_sb[:])
                # columns m = p*CD + c, p ascending -> out cols c::CD
                nc.vector.scalar_tensor_tensor(
                    out=out_sb[:, c:MDVE:CD], in0=tp[:], scalar=1.0,
                    in1=bias_dve_sb[:, c:MDVE:CD], op0=mult, op1=add)
            nc.sync.dma_start(out_d[:, 0:MDVE], out_sb[:, 0:MDVE])

    nc.compile()
    return nc


def _get_nc():
    if "nc" not in _CACHE:
        _CACHE["nc"] = _build_nc()
    return _CACHE["nc"]


def _host_prep_x(inputs):
    """Full X [B, M, I] f32 -> per-core feeds (x_pe packed, x_dve p-major)."""
    x16 = np.asarray(inputs, dtype=np.float16)
    feeds = []
    for core in range(NCORES):
        slab = x16[core * BLOC:(core + 1) * BLOC]      # [128 b, M, I]
        # PE part: m in [MDVE, M) -> [k, i, b, ml]
        pe = slab[:, MDVE:, :]                         # [b, MPE, i]
        pe = pe.reshape(BLOC, KCH, MCP, I)             # [b, k, ml, i]
        pe = np.ascontiguousarray(pe.transpose(1, 3, 0, 2))  # [k, i, b, ml]
        # DVE part: m in [0, MDVE), m = p*CD + c -> [p, b, c, i]
        dv = slab[:, :MDVE, :].reshape(BLOC, 128, CD, I)     # [b, p, c, i]
        dv = np.ascontiguousarray(dv.transpose(1, 0, 2, 3))  # [p, b, c, i]
        feeds.append((pe, dv))
    return feeds


def _host_prep_wb(Rk_weight, bias):
    w16 = np.asarray(Rk_weight, dtype=np.float16)      # [M, I]
    b32 = np.asarray(bias, dtype=np.float32)           # [M]
    wt_pe = np.ascontiguousarray(w16[MDVE:, :].T)      # [I, MPE]
    w_dve = np.ascontiguousarray(
        w16[:MDVE, :].reshape(128, CD, I))             # [p, c, i]
    bias_pe = np.ascontiguousarray(
        np.broadcast_to(b32[MDVE:], (128, MPE)))       # [128, MPE]
    bias_dve = np.ascontiguousarray(
        np.broadcast_to(b32[:MDVE], (128, MDVE)))      # [128, MDVE]
    return wt_pe, w_dve, bias_pe, bias_dve


def _sim_feeds(inputs):
    """Feeds for a core-0 CoreSim run (used by test.py --sim)."""
    (pe, dv) = _host_prep_x(np.asarray(inputs["inputs"]))[0]
    wt_pe, w_dve, bias_pe, bias_dve = _host_prep_wb(
        inputs["Rk_weight"], inputs["bias"])
    return {"x_pe": pe, "x_dve": dv, "wt_pe": wt_pe, "w_dve": w_dve,
            "bias_pe": bias_pe, "bias_dve": bias_dve}


SIM_TOL = 5e-3  # bf16 DVE path dominates: expect ~2e-3


def kernel(inputs, Rk_weight, bias):
    global LAST_RESULT
    _ensure_ntff_hook()
    from concourse.bass_utils import run_bass_kernel_spmd

    nc = _get_nc()

    xfeeds = _host_prep_x(np.asarray(inputs))
    wt_pe, w_dve, bias_pe, bias_dve = _host_prep_wb(Rk_weight, bias)

    in_maps = []
    for core in range(NCORES):
        pe, dv = xfeeds[core]
        in_maps.append({
            "x_pe": pe, "x_dve": dv, "wt_pe": wt_pe, "w_dve": w_dve,
            "bias_pe": bias_pe, "bias_dve": bias_dve,
        })

    res = run_bass_kernel_spmd(nc, in_maps, list(range(NCORES)))
    LAST_RESULT = res
    out = np.concatenate(
        [np.asarray(res.results[i]["out"]) for i in range(NCORES)], axis=0)
    return out.astype(np.float32, copy=False)
